# revision 32
# baseline (speedup 1.0000x reference)
"""Deformable conv (DCNv2-style) TRN2 Bass kernel.

Problem: x[8,64,128,128] f32; offset conv (27ch 3x3) -> (dy,dx,mask) per 9 taps;
bilinear sampling of x at tap positions + offsets; modulated; 3x3 conv via
per-tap 1x1 matmuls.

Strategy (per core, data-parallel over batch, 8 cores):
  - om conv: 9 shifted bf16 matmuls on zero-padded XP -> PSUM [27,512] chunks
    -> redistribute to OMT [h-part, j, w] via DMA.
  - stage2 (DVE/ACT on [128h, 9t, 128w]): floor/clip/hat-weights/mask/sigmoid
    -> W4C (bf16 corner-weight pairs, persistent), idx (uint16, u32-pair units
    into a 16-row window of X3, rebased by rowbase(m) = clamp(4m-6, 0, 112)).
  - X3: interleaved row-pair layout X3[c, r, 2j]=x[r,j], [.., 2j+1]=x[r+1,j]
    (bf16, viewed as u32 pairs); partitions 64-127 hold X3 shifted by one
    u32 (the x0+1 corners).  ONE indirect_copy per 512-pos chunk gathers all
    9 taps x 512 positions x (y0,y0+1)-pair as u32 elements from the 16-row
    window -> 4 corners for all 64 channels x 2 column-halves.
  - weights: per chunk, 8 SBUF->SBUF broadcast DMAs replicate W4C[4m+q, half]
    lines across the 64 channel partitions of each half.
  - P = G * Wrep (DVE bf16, in-place into Wrep); einsum: per chunk 9 taps x
    2 slots accumulating matmuls (lhsT = w_conv tap slices stacked x2) into
    PSUM [64,512] -> out.

Gather stream order: position s = 512*m + col*16 + p  (IDENTITY: s = 512m + i).
"""
import numpy as np
import ml_dtypes

from concourse.bacc import Bacc
from concourse import mybir, tile
from concourse.bass_utils import run_bass_kernel_spmd

np_bf16 = np.dtype(ml_dtypes.bfloat16)
f32 = mybir.dt.float32
bf16 = mybir.dt.bfloat16
u16 = mybir.dt.uint16
i16 = mybir.dt.int16
u32 = mybir.dt.uint32
i32 = mybir.dt.int32

B, C, H, W = 8, 64, 128, 128
HW = H * W          # 16384
T = 9               # taps
NJ = 27             # offset-conv channels
NCHUNK = 32         # 512-position chunks
CHUNK = 512
WIN_ROWS = 16       # gather window rows (u32 units per row = 128)
WIN = WIN_ROWS * 128
AF = mybir.ActivationFunctionType
ALU = mybir.AluOpType

_CACHE = {}


def _rowbase(m):
    return min(112, max(0, 4 * m - 6))


def _host_consts():
    # CYK[h, t] = h + ky(t) - 1 ; CXW[h, t, w] = w + kx(t) - 1 (h-independent)
    ky = np.arange(T) // 3
    kx = np.arange(T) % 3
    cyk = (np.arange(128)[:, None] + ky[None, :] - 1).astype(np.float32)
    cxw = np.broadcast_to(
        (np.arange(128)[None, :] + kx[:, None] - 1)[None, :, :], (128, T, 128)
    ).astype(np.float32).copy()
    # NEGRB[h] = -128 * rowbase(h//4): idx rebase for the 16-row window
    negrb = np.array([-128.0 * _rowbase(h // 4) for h in range(128)],
                     dtype=np.float32).reshape(128, 1)
    return cyk, cxw, negrb


def build_nc(num_devices=8, debug=False):
    """Build the per-core SPMD kernel. debug: also emit debug tensors."""
    nc = Bacc("TRN2", target_bir_lowering=False, debug=False,
              num_devices=num_devices)

    x_in = nc.dram_tensor("x_in", [C, HW], f32, kind="ExternalInput")
    woffT_in = nc.dram_tensor("woffT_in", [C, T * NJ], f32, kind="ExternalInput")
    boff_in = nc.dram_tensor("boff_in", [NJ, 1], f32, kind="ExternalInput")
    wk2_in = nc.dram_tensor("wk2_in", [128, T * C], bf16, kind="ExternalInput")
    out_dram = nc.dram_tensor("out", [C, HW], f32, kind="ExternalOutput")

    cyk_np, cxw_np, negrb_np = _host_consts()
    cyk_const = nc.inline_tensor(cyk_np, name="cyk_const")
    cxw_const = nc.inline_tensor(cxw_np.reshape(128, T * 128), name="cxw_const")
    negrb_const = nc.inline_tensor(negrb_np, name="negrb_const")

    with tile.TileContext(nc) as tc:
        with tc.tile_pool(name="main", bufs=1) as mp, \
             tc.tile_pool(name="dram", bufs=1, space="DRAM") as drp, \
             tc.tile_pool(name="dbuf", bufs=2) as dbp:
            # ---------- persistent tiles ----------
            X3 = mp.tile([128, HW * 2], bf16, tag="X3")          # 64 KiB/part
            IDXT2 = mp.tile([128, NCHUNK, T, 32], i16, tag="IDXT2")
            W4C = mp.tile([128, 2, T, 128, 2], bf16, tag="W4C")  # [h,half,t,w,slot]
            WLIN = drp.tile([NCHUNK, 2, T, CHUNK, 2], bf16, tag="WLIN")
            WLIN2 = drp.tile([128, 2, T, 128, 2], bf16, tag="WLIN2")
            OMD = drp.tile([NJ, HW], f32, tag="OMD")
            IDXD2 = drp.tile([16, T, 128, 8], i16, tag="IDXD2")
            CYK = mp.tile([128, T], f32, tag="CYK")
            CXW = mp.tile([128, T, 128], f32, tag="CXW")
            NEGRB = mp.tile([128, 1], f32, tag="NEGRB")
            WOFFT = mp.tile([C, T, NJ], bf16, tag="WOFFT")
            WK2 = mp.tile([128, T, C], bf16, tag="WK2")
            BOFF = mp.tile([NJ, 1], f32, tag="BOFF")

            nc.sync.dma_start(CYK[:], cyk_const.ap())
            nc.sync.dma_start(CXW[:].rearrange("p a b -> p (a b)"), cxw_const.ap())
            nc.sync.dma_start(NEGRB[:], negrb_const.ap())
            nc.sync.dma_start(WK2[:].rearrange("p a b -> p (a b)"), wk2_in.ap())
            nc.sync.dma_start(BOFF[:], boff_in.ap())

            # preload activation-function tables during the x load
            DDA = mp.tile([128, 1], f32, tag="DDA")
            nc.scalar.activation(out=DDA[:], in_=CYK[:, 0:1], func=AF.Sigmoid)
            nc.scalar.activation(out=DDA[:], in_=DDA[:], func=AF.Abs)
            nc.scalar.activation(out=DDA[:], in_=DDA[:], func=AF.Relu)

            # ================= Phase A: pad + X3 + om conv =================
            midcm = tc.tile_pool(name="mid", bufs=1)
            midp = midcm.__enter__()
            OMT = midp.tile([128, NJ, 128], f32, tag="OMT")      # [h, j, w]
            IDXF = midp.tile([128, T, 16, 8], i16, tag="IDXF")   # [h,t,b,a] (w=16a+b)
            with tc.tile_pool(name="early", bufs=1) as ep, \
                 tc.tile_pool(name="ompsum", bufs=3, space="PSUM") as opp:
                XP = ep.tile([C, 130 * 130], bf16, tag="XP")
                WOFFS = ep.tile([C, T * NJ], f32, tag="WOFFS")
                nc.sync.dma_start(WOFFS[:], woffT_in.ap())
                nc.vector.tensor_copy(out=WOFFT[:].rearrange("p a b -> p (a b)"),
                                      in_=WOFFS[:])

                XP3 = XP[:].rearrange("p (r c2) -> p r c2", c2=130)
                # zero only the pad border (top/bottom rows, left/right cols)
                nc.vector.memset(XP3[:, 0, :], 0.0)
                nc.vector.memset(XP3[:, 129, :], 0.0)
                nc.vector.memset(XP3[:, 1:129, 0], 0.0)
                nc.vector.memset(XP3[:, 1:129, 129], 0.0)
                # staged x load: f32 -> bf16 in 4 row blocks
                with tc.tile_pool(name="xld", bufs=2) as xlp:
                    for blk in range(4):
                        XRT = xlp.tile([C, 32 * 128], f32, tag="XRT")
                        nc.sync.dma_start(
                            out=XRT[:],
                            in_=x_in.ap()[:, blk * 4096:(blk + 1) * 4096])
                        nc.scalar.copy(
                            out=XP3[:, 1 + 32 * blk:1 + 32 * (blk + 1), 1:129],
                            in_=XRT[:].rearrange("p (r w) -> p r w", w=128))

                # X3 A-half: X3[c, r*256 + 2j + s] = x[c, r+s, j]
                X3A = X3[0:64, :].rearrange("p (r j s) -> p r j s", j=128, s=2)
                nc.scalar.copy(out=X3A[:, :, :, 0], in_=XP3[0:64, 1:129, 1:129])
                nc.vector.tensor_copy(out=X3A[:, :, :, 1], in_=XP3[0:64, 2:130, 1:129])
                # B-half: shift by one u32 pair (cross-partition copy via DMA)
                nc.sync.dma_start(out=X3[64:128, 0:2 * HW - 2], in_=X3[0:64, 2:2 * HW])
                nc.vector.memset(X3[64:128, 2 * HW - 2:2 * HW], 0.0)

                # om conv: per 512-pos chunk, 9 accumulating bf16 matmuls.
                # Evictions land in OMS (8 chunks), then one DMA to DRAM and
                # one strided-read DMA per 32-row group into OMT [h, j, w].
                for g in range(4):
                    OMS = ep.tile([NJ, 8 * CHUNK], f32, tag="OMS",
                                  name=f"OMS_{g}")
                    for mi in range(8):
                        m = 8 * g + mi
                        ps = opp.tile([NJ, CHUNK], f32, tag="omps")
                        for t9 in range(T):
                            ty, tx = divmod(t9, 3)
                            rhs = XP3[0:64, 4 * m + ty: 4 * m + ty + 4,
                                      tx: tx + 128]
                            nc.tensor.matmul(ps[:], lhsT=WOFFT[:, t9, :],
                                             rhs=rhs, start=(t9 == 0),
                                             stop=(t9 == T - 1))
                        nc.scalar.activation(
                            out=OMS[:, mi * CHUNK:(mi + 1) * CHUNK], in_=ps[:],
                            func=AF.Identity, bias=BOFF[:])
                    eng = nc.sync if (g % 2 == 0) else nc.scalar
                    eng.dma_start(out=OMD[:, g * 4096:(g + 1) * 4096],
                                  in_=OMS[:])
                    # OMT[32g + i, j, w] = OMD[j, (32g+i)*128 + w]
                    eng2 = nc.scalar if (g % 2 == 0) else nc.sync
                    eng2.dma_start(
                        out=OMT[32 * g:32 * (g + 1), :, :],
                        in_=OMD[:, g * 4096:(g + 1) * 4096]
                            .rearrange("j (i w) -> i j w", w=128))
            # ================= stage 2: weights + idx =================
            with tc.tile_pool(name="s2", bufs=1) as sp:
                OMTv = OMT[:]
                DY = OMTv[:, 0:18, :].rearrange("p (k s) w -> p k s w", s=2)[:, :, 0, :]
                DX = OMTv[:, 0:18, :].rearrange("p (k s) w -> p k s w", s=2)[:, :, 1, :]
                MS = OMTv[:, 18:27, :]

                sh = [128, T, 128]
                YS = sp.tile(sh, f32, tag="YS")
                XS = sp.tile(sh, f32, tag="XS")
                Y0C = sp.tile(sh, f32, tag="Y0C")
                X0C = sp.tile(sh, f32, tag="X0C")
                TMPI = sp.tile(sh, i32, tag="TMPI")
                TY = sp.tile(sh, f32, tag="TY")
                TX = sp.tile(sh, f32, tag="TX")
                WYA = sp.tile(sh, f32, tag="WYA")
                WYB = sp.tile(sh, f32, tag="WYB")
                WXA = sp.tile(sh, f32, tag="WXA")
                WXB = sp.tile(sh, f32, tag="WXB")
                MSK = sp.tile(sh, f32, tag="MSK")
                TMP = sp.tile(sh, f32, tag="TMP")
                TMP2 = sp.tile(sh, f32, tag="TMP2")

                CYKb = CYK[:].unsqueeze(2).broadcast_to(sh)

                # ys/xs
                nc.vector.tensor_tensor(out=YS[:], in0=DY, in1=CYKb, op=ALU.add)
                nc.vector.tensor_tensor(out=XS[:], in0=DX, in1=CXW[:], op=ALU.add)
                # floor: int-convert (any rounding mode) then fix up c > x
                for SRC, DSTF in ((YS, Y0C), (XS, X0C)):
                    nc.vector.tensor_copy(out=TMPI[:], in_=SRC[:])
                    nc.vector.tensor_copy(out=DSTF[:], in_=TMPI[:])
                    nc.vector.tensor_tensor(out=TMP[:], in0=DSTF[:], in1=SRC[:],
                                            op=ALU.is_gt)
                    nc.vector.tensor_tensor(out=DSTF[:], in0=DSTF[:], in1=TMP[:],
                                            op=ALU.subtract)
                    # clip to [0, 127]
                    nc.vector.tensor_scalar(out=DSTF[:], in0=DSTF[:], scalar1=0.0,
                                            scalar2=127.0, op0=ALU.max, op1=ALU.min)
                # t = s - clip ; weights
                nc.vector.tensor_tensor(out=TY[:], in0=YS[:], in1=Y0C[:], op=ALU.subtract)
                nc.vector.tensor_tensor(out=TX[:], in0=XS[:], in1=X0C[:], op=ALU.subtract)
                # wA = relu(1 - |t|), wBr = relu(t)
                nc.scalar.activation(out=TMP[:], in_=TY[:], func=AF.Abs)
                nc.scalar.activation(out=WYA[:], in_=TMP[:], func=AF.Relu, scale=-1.0, bias=1.0)
                nc.scalar.activation(out=WYB[:], in_=TY[:], func=AF.Relu)
                nc.scalar.activation(out=TMP2[:], in_=TX[:], func=AF.Abs)
                nc.scalar.activation(out=WXA[:], in_=TMP2[:], func=AF.Relu, scale=-1.0, bias=1.0)
                nc.scalar.activation(out=WXB[:], in_=TX[:], func=AF.Relu)
                # upper-boundary masks: wyB *= (ys < 127); wxB *= (xs < 127)
                nc.vector.tensor_scalar(out=TMP[:], in0=YS[:], scalar1=127.0,
                                        scalar2=None, op0=ALU.is_lt)
                nc.vector.tensor_tensor(out=WYB[:], in0=WYB[:], in1=TMP[:], op=ALU.mult)
                nc.vector.tensor_scalar(out=TMP2[:], in0=XS[:], scalar1=127.0,
                                        scalar2=None, op0=ALU.is_lt)
                nc.vector.tensor_tensor(out=WXB[:], in0=WXB[:], in1=TMP2[:], op=ALU.mult)
                # mask; fold into wx
                nc.scalar.activation(out=MSK[:], in_=MS, func=AF.Sigmoid)
                nc.vector.tensor_tensor(out=WXA[:], in0=WXA[:], in1=MSK[:], op=ALU.mult)
                nc.vector.tensor_tensor(out=WXB[:], in0=WXB[:], in1=MSK[:], op=ALU.mult)
                # products -> W4C (bf16, interleaved)
                nc.vector.tensor_tensor(out=W4C[:, 0, :, :, 0], in0=WYA[:], in1=WXA[:], op=ALU.mult)
                nc.vector.tensor_tensor(out=W4C[:, 0, :, :, 1], in0=WYB[:], in1=WXA[:], op=ALU.mult)
                nc.vector.tensor_tensor(out=W4C[:, 1, :, :, 0], in0=WYA[:], in1=WXB[:], op=ALU.mult)
                nc.vector.tensor_tensor(out=W4C[:, 1, :, :, 1], in0=WYB[:], in1=WXB[:], op=ALU.mult)
                # idx (u32-pair units into the 16-row window):
                #   idx = y0*128 + x0 - 128*rowbase(h//4)
                nc.vector.scalar_tensor_tensor(
                    out=TMP[:], in0=Y0C[:], scalar=128.0, in1=X0C[:],
                    op0=ALU.mult, op1=ALU.add)
                nc.scalar.activation(out=TMP2[:], in_=TMP[:], func=AF.Identity,
                                     bias=NEGRB[:])
                # out enumeration must follow in (t, w) = (t, a, b); IDXF
                # stores [h, t, b, a] so permute the out AP
                nc.vector.tensor_copy(
                    out=IDXF[:].rearrange("p t b2 a -> p t a b2"),
                    in_=TMP2[:].rearrange("p t (a b2) -> p t a b2", a=8))

                if debug:
                    d_w4 = nc.dram_tensor("d_w4", [128, 2 * T * 128 * 2], bf16, kind="ExternalOutput")
                    d_idx = nc.dram_tensor("d_idx", [128, T * 8 * 16], i16, kind="ExternalOutput")
                    d_omt = nc.dram_tensor("d_omt", [128, NJ * 128], f32, kind="ExternalOutput")
                    d_y0c = nc.dram_tensor("d_y0c", [128, T * 128], f32, kind="ExternalOutput")
                    d_x0c = nc.dram_tensor("d_x0c", [128, T * 128], f32, kind="ExternalOutput")
                    nc.sync.dma_start(d_w4.ap(), W4C[:].rearrange("p a b c d -> p (a b c d)"))
                    nc.sync.dma_start(d_idx.ap(), IDXF[:].rearrange("p a b c -> p (a b c)"))
                    nc.sync.dma_start(d_omt.ap(), OMT[:].rearrange("p a b -> p (a b)"))
                    nc.sync.dma_start(d_y0c.ap(), Y0C[:].rearrange("p a b -> p (a b)"))
                    nc.sync.dma_start(d_x0c.ap(), X0C[:].rearrange("p a b -> p (a b)"))

            # ---------- IDXT2 build: 16 + 1 DMAs + 3 replications ----------
            # IDXT2[p, m, t, col] = idx(s = 512m + col*16 + p, tap t)
            #                     = IDXF[h = 4m + col//8, t, a = col%8, b = p]
            # hop1: IDXD2[b, t, h, a] = IDXF[h, t, b, a]  (16 DMAs to DRAM)
            ID2v = IDXD2[:]
            for p16 in range(16):
                eng = nc.sync if (p16 % 2 == 0) else nc.scalar
                eng.dma_start(
                    out=ID2v[p16:p16 + 1, :, :, :].rearrange(
                        "o t h a -> o h t a"),
                    in_=IDXF[:, :, p16, :])
            # hop2: IDXT2[b, m, t, (hl a)] = IDXD2[b, t, 4m+hl, a]  (9 DMAs)
            for t9 in range(T):
                eng = nc.sync if (t9 % 2 == 0) else nc.scalar
                eng.dma_start(
                    out=IDXT2[0:16, :, t9, :],
                    in_=ID2v[:, t9, :, :].rearrange(
                        "b (m hl) a -> b m (hl a)", hl=4))
            # replicate partitions 0-15 -> 16-127 (doubling)
            nc.sync.dma_start(out=IDXT2[16:32], in_=IDXT2[0:16])
            nc.sync.dma_start(out=IDXT2[32:64], in_=IDXT2[0:32])
            nc.sync.dma_start(out=IDXT2[64:128], in_=IDXT2[0:64])

            # ---------- WLIN build: contiguous dump + 8 DRAM reorders ------
            # WLIN[m, half, t, hl*128 + w, slot] = W4C[4m+hl, half, t, w, slot]
            WLINv = WLIN[:]
            W2v = WLIN2[:]
            nc.sync.dma_start(
                out=W2v.rearrange("p a b c d -> p (a b c d)"),
                in_=W4C[:].rearrange("p a b c d -> p (a b c d)"))
            for half in range(2):
                for hl in range(4):
                    eng = nc.sync if (hl % 2 == 0) else nc.scalar
                    eng.dma_start(
                        out=WLINv[:, half, :, hl * 128:(hl + 1) * 128, :],
                        in_=W2v[hl:128:4, half, :, :, :])
            midcm.__exit__(None, None, None)

            # ================= main loop =================
            X3u = X3[:].bitcast(u32)                  # [128, 16384] u32 pairs
            with tc.tile_pool(name="gl", bufs=2) as gp, \
                 tc.tile_pool(name="wl", bufs=3) as wp, \
                 tc.tile_pool(name="ps2", bufs=2, space="PSUM") as pp2:
                # absorb initial deps into gpsimd queue
                dd1 = mp.tile([128, 1], bf16, tag="dd1")
                dd2 = mp.tile([128, 1], i16, tag="dd2")
                nc.gpsimd.tensor_copy(out=dd1[:], in_=X3[:, 0:1])
                nc.gpsimd.tensor_copy(out=dd2[:], in_=IDXT2[:, 0, 0, 0:1])

                for m in range(NCHUNK):
                    rb = _rowbase(m)
                    GQ = gp.tile([128, T * CHUNK], u32, tag="GQ")
                    WRQ = wp.tile([128, T, CHUNK, 2], bf16, tag="WRQ")
                    # gather: all 9 taps for this chunk from the 16-row window
                    nc.gpsimd.ap_gather(
                        out_ap=GQ[:].unsqueeze(2),
                        in_ap=X3u[:, rb * 128: rb * 128 + WIN].unsqueeze(2),
                        idxs_ap=IDXT2[:, m].rearrange("p t c -> p (t c)"),
                        channels=128, num_elems=WIN, d=1,
                        num_idxs=T * CHUNK)
                    # weight broadcast: one DMA per column-half (from DRAM)
                    for half in range(2):
                        eng = nc.sync if (half == 0) else nc.scalar
                        eng.dma_start(
                            out=WRQ[64 * half:64 * (half + 1)],
                            in_=WLINv[m, half].unsqueeze(0)
                                .broadcast_to([64, T, CHUNK, 2]))
                    if debug and m == 0:
                        d_gq = nc.dram_tensor("d_gq", [128, T * CHUNK], u32, kind="ExternalOutput")
                        d_wrq = nc.dram_tensor("d_wrq", [128, T * CHUNK * 2], bf16, kind="ExternalOutput")
                        nc.sync.dma_start(d_gq.ap(), GQ[:])
                        nc.sync.dma_start(d_wrq.ap(), WRQ[:].rearrange("p a b c -> p (a b c)"))
                    # modulate: P = G * Wrep (in-place into WRQ)
                    WRQf = WRQ[:].rearrange("p a b c -> p (a b c)")
                    nc.vector.tensor_tensor(
                        out=WRQf, in0=GQ[:].bitcast(bf16), in1=WRQf, op=ALU.mult)
                    # einsum: accumulate 9 taps x 2 slots into PSUM [64, 512]
                    ps = pp2.tile([C, CHUNK], f32, tag="eps")
                    for tap in range(T):
                        for slot in range(2):
                            nc.tensor.matmul(
                                ps[:], lhsT=WK2[:, tap, :],
                                rhs=WRQ[:, tap, :, slot],
                                start=(tap == 0 and slot == 0),
                                stop=(tap == T - 1 and slot == 1))
                    osb = dbp.tile([C, CHUNK], f32, tag="osb")
                    nc.scalar.copy(out=osb[:], in_=ps[:])
                    eng = nc.sync if (m % 2 == 0) else nc.scalar
                    eng.dma_start(
                        out=out_dram.ap()[:, m * CHUNK:(m + 1) * CHUNK],
                        in_=osb[:])

    nc.compile()
    return nc


def _prep_weights(w_offset, b_offset, w_conv):
    w_offset = np.asarray(w_offset, dtype=np.float32)
    w_conv = np.asarray(w_conv, dtype=np.float32)
    b_offset = np.asarray(b_offset, dtype=np.float32)
    # woffT[c, t*27 + j] = w_offset[j, c, ty, tx]
    woffT = w_offset.transpose(2, 3, 1, 0).reshape(T, C, NJ)  # [t, c, j]
    woffT = woffT.transpose(1, 0, 2).reshape(C, T * NJ).copy()
    boff = b_offset.reshape(NJ, 1).copy()
    # wk2[q, t*64 + o] = w_conv[o, q%64, ty, tx]
    wkt = w_conv.transpose(2, 3, 1, 0).reshape(T, C, C)       # [t, c, o]
    wk2 = np.concatenate([wkt, wkt], axis=1)                   # [t, 128, o]
    wk2 = wk2.transpose(1, 0, 2).reshape(128, T * C).astype(np_bf16).copy()
    return woffT, boff, wk2


def _get_runner():
    """Build a persistent jitted shard_map runner for the cached nc (avoids
    per-call retracing that run_bass_via_pjrt pays)."""
    if "runner" in _CACHE:
        return _CACHE["runner"]
    import jax
    import jax.numpy as jnp
    from jax.sharding import Mesh, PartitionSpec, NamedSharding
    from jax.experimental.shard_map import shard_map
    from concourse import bass2jax

    bass2jax.install_neuronx_cc_hook()
    nc = _CACHE["nc"]
    partition_name = (nc.partition_id_tensor.name
                      if nc.partition_id_tensor else None)
    in_names, out_names, out_avals, zero_shapes = [], [], [], []
    for alloc in nc.m.functions[0].allocations:
        if not isinstance(alloc, mybir.MemoryLocationSet):
            continue
        name = alloc.memorylocations[0].name
        if alloc.kind == "ExternalInput":
            if name != partition_name:
                in_names.append(name)
        elif alloc.kind == "ExternalOutput":
            out_names.append(name)
            shape = tuple(alloc.tensor_shape)
            dtype = mybir.dt.np(alloc.dtype)
            out_avals.append(jax.core.ShapedArray(shape, dtype))
            zero_shapes.append((shape, dtype))
    n_params = len(in_names)
    all_in = list(in_names) + list(out_names)
    if partition_name is not None:
        all_in.append(partition_name)
    donate = tuple(range(n_params, n_params + len(out_names)))

    def _body(*args):
        operands = list(args)
        if partition_name is not None:
            operands.append(bass2jax.partition_id_tensor())
        return tuple(bass2jax._bass_exec_p.bind(
            *operands,
            out_avals=tuple(out_avals),
            in_names=tuple(all_in),
            out_names=tuple(out_names),
            lowering_input_output_aliases=(),
            sim_require_finite=True,
            sim_require_nnan=True,
            nc=nc,
        ))

    devices = jax.devices()[:B]
    mesh = Mesh(np.asarray(devices), ("core",))
    in_specs = (PartitionSpec("core"),) * (n_params + len(out_names))
    out_specs = (PartitionSpec("core"),) * len(out_names)
    sharded = jax.jit(
        shard_map(_body, mesh=mesh, in_specs=in_specs, out_specs=out_specs,
                  check_rep=False),
        donate_argnums=donate, keep_unused=True)

    shardings = NamedSharding(mesh, PartitionSpec("core"))
    zeros_fn = jax.jit(
        lambda: tuple(jnp.zeros((B * s[0], *s[1:]), d) for s, d in zero_shapes),
        out_shardings=(shardings,) * len(zero_shapes))

    info = dict(sharded=sharded, zeros_fn=zeros_fn, in_names=in_names,
                out_names=out_names, mesh=mesh, shardings=shardings)
    _CACHE["runner"] = info
    return info


def _concat_inputs(x, woffT, boff, wk2):
    # per-core inputs concatenated along axis 0 (cores share the weights)
    xs = np.ascontiguousarray(x.reshape(B * C, HW))
    return {
        "x_in": xs,
        "woffT_in": np.tile(woffT, (B, 1)),
        "boff_in": np.tile(boff, (B, 1)),
        "wk2_in": np.tile(wk2, (B, 1)),
    }


def kernel(x, w_offset, b_offset, w_conv):
    x = np.asarray(x, dtype=np.float32)
    woffT, boff, wk2 = _prep_weights(w_offset, b_offset, w_conv)
    if "nc" not in _CACHE:
        _CACHE["nc"] = build_nc(num_devices=B)
    r = _get_runner()
    cin = _concat_inputs(x, woffT, boff, wk2)
    args = [cin[n] for n in r["in_names"]]
    zeros = r["zeros_fn"]()
    outs = r["sharded"](*args, *zeros)
    out = np.asarray(outs[r["out_names"].index("out")])
    return out.reshape(B, C, H, W).astype(np.float32)


def bench_exec_ns(inp, reps=16):
    """Amortized per-invocation device time: pipelined repeats with staged
    device inputs (excludes host prep + H2D of the big inputs)."""
    import time
    import jax
    x = np.asarray(inp["x"], dtype=np.float32)
    woffT, boff, wk2 = _prep_weights(inp["w_offset"], inp["b_offset"],
                                     inp["w_conv"])
    if "nc" not in _CACHE:
        _CACHE["nc"] = build_nc(num_devices=B)
    r = _get_runner()
    cin = _concat_inputs(x, woffT, boff, wk2)
    args_host = [cin[n] for n in r["in_names"]]
    args_dev = [jax.device_put(a, r["shardings"]) for a in args_host]
    jax.block_until_ready(args_dev)
    oi = r["out_names"].index("out")
    # warm
    o = r["sharded"](*args_dev, *r["zeros_fn"]())
    jax.block_until_ready(o)
    best = None
    for _ in range(3):
        zs = [r["zeros_fn"]() for _ in range(reps)]
        jax.block_until_ready(zs)
        t0 = time.perf_counter()
        outs = [r["sharded"](*args_dev, *z) for z in zs]
        jax.block_until_ready(outs[-1][oi])
        dt = (time.perf_counter() - t0) / reps
        best = dt if best is None else min(best, dt)
    return best * 1e9


# revision 36
# speedup vs baseline: 1.2600x; 1.2600x over previous
"""Deformable conv (DCNv2-style) TRN2 Bass kernel.

Problem: x[8,64,128,128] f32; offset conv (27ch 3x3) -> (dy,dx,mask) per 9 taps;
bilinear sampling of x at tap positions + offsets; modulated; 3x3 conv via
per-tap 1x1 matmuls.

Strategy (per core, data-parallel over batch, 8 cores):
  - om conv: 9 shifted bf16 matmuls on zero-padded XP -> PSUM [27,512] chunks
    -> OMS staging -> DRAM -> OMT [h-part, j, w].
  - stage2 (DVE/ACT on [128h, 9t, 128w]): rounding-mode-agnostic floor /clip/
    hat-weights/mask/sigmoid -> W4C [h, ry, rx, t, w] (bf16), idx (int16,
    quad-unit index y0*128+x0 into XQ).
  - XQ (DRAM): quad image XQ[pos, (rx,ry), c] = x[c, pos + rx + 128*ry],
    built on-device: PE row transposes -> XTP[pos, c] (DRAM) -> 4 shifted
    DRAM->DRAM copies. One dma_gather(transpose=True, elem 512B) per chunk
    fetches all 9 taps x 512 positions x 4 corners on the 16 SDMA engines;
    out lands as [128=(c,ry), rx, i] -- the matmul contraction layout.
  - weights: WLIN5[m, ry, rx, t, s] (DRAM); per chunk two broadcast DMAs
    replicate across the 64 channel partitions of each ry half.
  - P = G * Wrep (DVE bf16, in-place); einsum: per chunk 9 taps x 2 rx
    accumulating matmuls (contraction (c, ry)) into PSUM [64,512] -> out.

Gather stream order: position s = 512*m + col*16 + p  (IDENTITY: s = 512m + i).
"""
import numpy as np
import ml_dtypes

from concourse.bacc import Bacc
from concourse import mybir, tile
from concourse.bass_utils import run_bass_kernel_spmd

np_bf16 = np.dtype(ml_dtypes.bfloat16)
f32 = mybir.dt.float32
bf16 = mybir.dt.bfloat16
i16 = mybir.dt.int16
u32 = mybir.dt.uint32
i32 = mybir.dt.int32

B, C, H, W = 8, 64, 128, 128
HW = H * W          # 16384
T = 9               # taps
NJ = 27             # offset-conv channels
NCHUNK = 32         # 512-position chunks
CHUNK = 512
XTPR = HW + 256     # XTP rows incl zero pad (max idx + 129)
AF = mybir.ActivationFunctionType
ALU = mybir.AluOpType

_CACHE = {}


def _host_consts():
    # CYK[h, t] = h + ky(t) - 1 ; CXW[h, t, w] = w + kx(t) - 1 (h-independent)
    ky = np.arange(T) // 3
    kx = np.arange(T) % 3
    cyk = (np.arange(128)[:, None] + ky[None, :] - 1).astype(np.float32)
    cxw = np.broadcast_to(
        (np.arange(128)[None, :] + kx[:, None] - 1)[None, :, :], (128, T, 128)
    ).astype(np.float32).copy()
    idn = np.eye(64, dtype=np_bf16)
    return cyk, cxw, idn


def build_nc(num_devices=8, debug=False):
    """Build the per-core SPMD kernel. debug: also emit debug tensors."""
    nc = Bacc("TRN2", target_bir_lowering=False, debug=False,
              num_devices=num_devices)

    x_in = nc.dram_tensor("x_in", [C, HW], f32, kind="ExternalInput")
    woffT_in = nc.dram_tensor("woffT_in", [C, T * NJ], f32, kind="ExternalInput")
    boff_in = nc.dram_tensor("boff_in", [NJ, 1], f32, kind="ExternalInput")
    wk2_in = nc.dram_tensor("wk2_in", [128, T * C], bf16, kind="ExternalInput")
    out_dram = nc.dram_tensor("out", [C, HW], f32, kind="ExternalOutput")

    cyk_np, cxw_np, idn_np = _host_consts()
    cyk_const = nc.inline_tensor(cyk_np, name="cyk_const")
    cxw_const = nc.inline_tensor(cxw_np.reshape(128, T * 128), name="cxw_const")
    idn_const = nc.inline_tensor(idn_np, name="idn_const")

    with tile.TileContext(nc) as tc:
        with tc.tile_pool(name="main", bufs=1) as mp, \
             tc.tile_pool(name="dram", bufs=1, space="DRAM") as drp, \
             tc.tile_pool(name="dbuf", bufs=2) as dbp:
            # ---------- persistent tiles ----------
            IDXT2 = mp.tile([128, NCHUNK, T, 32], i16, tag="IDXT2")
            W4C = mp.tile([128, 2, 2, T, 128], bf16, tag="W4C")  # [h,ry,rx,t,w]
            WLIN5 = drp.tile([NCHUNK, 2, T, 2, CHUNK], bf16, tag="WLIN5")
            WLIN2 = drp.tile([128, 2, 2, T, 128], bf16, tag="WLIN2")
            OMD = drp.tile([NJ, HW], f32, tag="OMD")
            IDXD2 = drp.tile([16, T, 128, 8], i16, tag="IDXD2")
            XTP = drp.tile([XTPR, C], bf16, tag="XTP")
            XQ = drp.tile([HW, 4, C], bf16, tag="XQ")
            CYK = mp.tile([128, T], f32, tag="CYK")
            CXW = mp.tile([128, T, 128], f32, tag="CXW")
            IDN = mp.tile([C, C], bf16, tag="IDN")
            WOFFT = mp.tile([C, T, NJ], bf16, tag="WOFFT")
            WK2 = mp.tile([128, T, C], bf16, tag="WK2")
            BOFF = mp.tile([NJ, 1], f32, tag="BOFF")
            ZT = mp.tile([128, C], bf16, tag="ZT")

            nc.sync.dma_start(CYK[:], cyk_const.ap())
            nc.sync.dma_start(CXW[:].rearrange("p a b -> p (a b)"), cxw_const.ap())
            nc.sync.dma_start(IDN[:], idn_const.ap())
            nc.sync.dma_start(WK2[:].rearrange("p a b -> p (a b)"), wk2_in.ap())
            nc.sync.dma_start(BOFF[:], boff_in.ap())
            nc.vector.memset(ZT[:], 0.0)

            # preload activation-function tables during the x load
            DDA = mp.tile([128, 1], f32, tag="DDA")
            nc.scalar.activation(out=DDA[:], in_=CYK[:, 0:1], func=AF.Sigmoid)
            nc.scalar.activation(out=DDA[:], in_=DDA[:], func=AF.Abs)
            nc.scalar.activation(out=DDA[:], in_=DDA[:], func=AF.Relu)

            # ================= Phase A: pad + om conv + XTP/XQ ============
            midcm = tc.tile_pool(name="mid", bufs=1)
            midp = midcm.__enter__()
            OMT = midp.tile([128, NJ, 128], f32, tag="OMT")      # [h, j, w]
            IDXF = midp.tile([128, T, 16, 8], i16, tag="IDXF")   # [h,t,b,a] (w=16a+b)
            with tc.tile_pool(name="early", bufs=1) as ep, \
                 tc.tile_pool(name="ompsum", bufs=3, space="PSUM") as opp, \
                 tc.tile_pool(name="tpsum", bufs=2, space="PSUM") as tpp:
                XP = ep.tile([C, 130 * 130], bf16, tag="XP")
                WOFFS = ep.tile([C, T * NJ], f32, tag="WOFFS")
                nc.sync.dma_start(WOFFS[:], woffT_in.ap())
                nc.vector.tensor_copy(out=WOFFT[:].rearrange("p a b -> p (a b)"),
                                      in_=WOFFS[:])

                XP3 = XP[:].rearrange("p (r c2) -> p r c2", c2=130)
                # zero only the pad border (top/bottom rows, left/right cols)
                nc.vector.memset(XP3[:, 0, :], 0.0)
                nc.vector.memset(XP3[:, 129, :], 0.0)
                nc.vector.memset(XP3[:, 1:129, 0], 0.0)
                nc.vector.memset(XP3[:, 1:129, 129], 0.0)
                # staged x load: f32 -> bf16 in 4 row blocks
                with tc.tile_pool(name="xld", bufs=2) as xlp:
                    for blk in range(4):
                        XRT = xlp.tile([C, 32 * 128], f32, tag="XRT")
                        nc.sync.dma_start(
                            out=XRT[:],
                            in_=x_in.ap()[:, blk * 4096:(blk + 1) * 4096])
                        nc.scalar.copy(
                            out=XP3[:, 1 + 32 * blk:1 + 32 * (blk + 1), 1:129],
                            in_=XRT[:].rearrange("p (r w) -> p r w", w=128))

                # ---- XTP: PE row transposes -> DRAM [pos, c] ----
                XTPa = XTP[:]
                for g16 in range(16):
                    TP = tpp.tile([128, 8, C], bf16, tag="TP")
                    for r in range(8):
                        h = 8 * g16 + r
                        nc.tensor.transpose(
                            out=TP[:, r, :],
                            in_=XP3[0:64, 1 + h, 1:129],
                            identity=IDN[:])
                    SG = dbp.tile([128, 8 * C], bf16, tag="SG")
                    nc.scalar.copy(out=SG[:],
                                   in_=TP[:].rearrange("p a b -> p (a b)"))
                    SGv = SG[:].rearrange("p (r c) -> p r c", c=C)
                    for r in range(8):
                        h = 8 * g16 + r
                        eng = nc.sync if (r % 2 == 0) else nc.scalar
                        eng.dma_start(out=XTPa[h * 128:(h + 1) * 128, :],
                                      in_=SGv[:, r, :])
                # zero pad rows HW..XTPR
                nc.sync.dma_start(out=XTPa[HW:HW + 128, :], in_=ZT[:])
                nc.scalar.dma_start(out=XTPa[HW + 128:HW + 256, :], in_=ZT[:])
                # ---- XQ assembly: 4 shifted DRAM->DRAM copies ----
                XQa = XQ[:]
                for rx in range(2):
                    for ry in range(2):
                        q = rx * 2 + ry
                        delta = rx + 128 * ry
                        eng = nc.sync if (q % 2 == 0) else nc.scalar
                        eng.dma_start(out=XQa[:, q, :],
                                      in_=XTPa[delta:delta + HW, :])

                # om conv: per 512-pos chunk, 9 accumulating bf16 matmuls.
                for g in range(4):
                    OMS = ep.tile([NJ, 8 * CHUNK], f32, tag="OMS",
                                  name=f"OMS_{g}")
                    for mi in range(8):
                        m = 8 * g + mi
                        ps = opp.tile([NJ, CHUNK], f32, tag="omps")
                        for t9 in range(T):
                            ty, tx = divmod(t9, 3)
                            rhs = XP3[0:64, 4 * m + ty: 4 * m + ty + 4,
                                      tx: tx + 128]
                            nc.tensor.matmul(ps[:], lhsT=WOFFT[:, t9, :],
                                             rhs=rhs, start=(t9 == 0),
                                             stop=(t9 == T - 1))
                        nc.scalar.activation(
                            out=OMS[:, mi * CHUNK:(mi + 1) * CHUNK], in_=ps[:],
                            func=AF.Identity, bias=BOFF[:])
                    eng = nc.sync if (g % 2 == 0) else nc.scalar
                    eng.dma_start(out=OMD[:, g * 4096:(g + 1) * 4096],
                                  in_=OMS[:])
                    # OMT[32g + i, j, w] = OMD[j, (32g+i)*128 + w]
                    eng2 = nc.scalar if (g % 2 == 0) else nc.sync
                    eng2.dma_start(
                        out=OMT[32 * g:32 * (g + 1), :, :],
                        in_=OMD[:, g * 4096:(g + 1) * 4096]
                            .rearrange("j (i w) -> i j w", w=128))
            # ================= stage 2: weights + idx =================
            with tc.tile_pool(name="s2", bufs=1) as sp:
                OMTv = OMT[:]
                DY = OMTv[:, 0:18, :].rearrange("p (k s) w -> p k s w", s=2)[:, :, 0, :]
                DX = OMTv[:, 0:18, :].rearrange("p (k s) w -> p k s w", s=2)[:, :, 1, :]
                MS = OMTv[:, 18:27, :]

                sh = [128, T, 128]
                YS = sp.tile(sh, f32, tag="YS")
                XS = sp.tile(sh, f32, tag="XS")
                Y0C = sp.tile(sh, f32, tag="Y0C")
                X0C = sp.tile(sh, f32, tag="X0C")
                TMPI = sp.tile(sh, i32, tag="TMPI")
                TY = sp.tile(sh, f32, tag="TY")
                TX = sp.tile(sh, f32, tag="TX")
                WYA = sp.tile(sh, f32, tag="WYA")
                WYB = sp.tile(sh, f32, tag="WYB")
                WXA = sp.tile(sh, f32, tag="WXA")
                WXB = sp.tile(sh, f32, tag="WXB")
                MSK = sp.tile(sh, f32, tag="MSK")
                TMP = sp.tile(sh, f32, tag="TMP")
                TMP2 = sp.tile(sh, f32, tag="TMP2")

                CYKb = CYK[:].unsqueeze(2).broadcast_to(sh)

                # ys/xs
                nc.vector.tensor_tensor(out=YS[:], in0=DY, in1=CYKb, op=ALU.add)
                nc.vector.tensor_tensor(out=XS[:], in0=DX, in1=CXW[:], op=ALU.add)
                # floor: int-convert (any rounding mode) then fix up c > x
                for SRC, DSTF in ((YS, Y0C), (XS, X0C)):
                    nc.vector.tensor_copy(out=TMPI[:], in_=SRC[:])
                    nc.vector.tensor_copy(out=DSTF[:], in_=TMPI[:])
                    nc.vector.tensor_tensor(out=TMP[:], in0=DSTF[:], in1=SRC[:],
                                            op=ALU.is_gt)
                    nc.vector.tensor_tensor(out=DSTF[:], in0=DSTF[:], in1=TMP[:],
                                            op=ALU.subtract)
                    # clip to [0, 127]
                    nc.vector.tensor_scalar(out=DSTF[:], in0=DSTF[:], scalar1=0.0,
                                            scalar2=127.0, op0=ALU.max, op1=ALU.min)
                # t = s - clip ; weights
                nc.vector.tensor_tensor(out=TY[:], in0=YS[:], in1=Y0C[:], op=ALU.subtract)
                nc.vector.tensor_tensor(out=TX[:], in0=XS[:], in1=X0C[:], op=ALU.subtract)
                # wA = relu(1 - |t|), wBr = relu(t)
                nc.scalar.activation(out=TMP[:], in_=TY[:], func=AF.Abs)
                nc.scalar.activation(out=WYA[:], in_=TMP[:], func=AF.Relu, scale=-1.0, bias=1.0)
                nc.scalar.activation(out=WYB[:], in_=TY[:], func=AF.Relu)
                nc.scalar.activation(out=TMP2[:], in_=TX[:], func=AF.Abs)
                nc.scalar.activation(out=WXA[:], in_=TMP2[:], func=AF.Relu, scale=-1.0, bias=1.0)
                nc.scalar.activation(out=WXB[:], in_=TX[:], func=AF.Relu)
                # upper-boundary masks: wyB *= (ys < 127); wxB *= (xs < 127)
                nc.vector.tensor_scalar(out=TMP[:], in0=YS[:], scalar1=127.0,
                                        scalar2=None, op0=ALU.is_lt)
                nc.vector.tensor_tensor(out=WYB[:], in0=WYB[:], in1=TMP[:], op=ALU.mult)
                nc.vector.tensor_scalar(out=TMP2[:], in0=XS[:], scalar1=127.0,
                                        scalar2=None, op0=ALU.is_lt)
                nc.vector.tensor_tensor(out=WXB[:], in0=WXB[:], in1=TMP2[:], op=ALU.mult)
                # mask; fold into wx
                nc.scalar.activation(out=MSK[:], in_=MS, func=AF.Sigmoid)
                nc.vector.tensor_tensor(out=WXA[:], in0=WXA[:], in1=MSK[:], op=ALU.mult)
                nc.vector.tensor_tensor(out=WXB[:], in0=WXB[:], in1=MSK[:], op=ALU.mult)
                # products -> W4C [h, ry, rx, t, w] (bf16)
                nc.vector.tensor_tensor(out=W4C[:, 0, 0], in0=WYA[:], in1=WXA[:], op=ALU.mult)
                nc.vector.tensor_tensor(out=W4C[:, 0, 1], in0=WYA[:], in1=WXB[:], op=ALU.mult)
                nc.vector.tensor_tensor(out=W4C[:, 1, 0], in0=WYB[:], in1=WXA[:], op=ALU.mult)
                nc.vector.tensor_tensor(out=W4C[:, 1, 1], in0=WYB[:], in1=WXB[:], op=ALU.mult)
                # idx (quad-unit index): idx = y0*128 + x0
                nc.vector.scalar_tensor_tensor(
                    out=TMP[:], in0=Y0C[:], scalar=128.0, in1=X0C[:],
                    op0=ALU.mult, op1=ALU.add)
                # out enumeration must follow in (t, w) = (t, a, b); IDXF
                # stores [h, t, b, a] so permute the out AP
                nc.vector.tensor_copy(
                    out=IDXF[:].rearrange("p t b2 a -> p t a b2"),
                    in_=TMP[:].rearrange("p t (a b2) -> p t a b2", a=8))

                if debug:
                    d_w4 = nc.dram_tensor("d_w4", [128, 2 * 2 * T * 128], bf16, kind="ExternalOutput")
                    d_idx = nc.dram_tensor("d_idx", [128, T * 16 * 8], i16, kind="ExternalOutput")
                    d_omt = nc.dram_tensor("d_omt", [128, NJ * 128], f32, kind="ExternalOutput")
                    nc.sync.dma_start(d_w4.ap(), W4C[:].rearrange("p a b c d -> p (a b c d)"))
                    nc.sync.dma_start(d_idx.ap(), IDXF[:].rearrange("p a b c -> p (a b c)"))
                    nc.sync.dma_start(d_omt.ap(), OMT[:].rearrange("p a b -> p (a b)"))

            # ---------- IDXT2 build: 16 + 9 DMAs + 3 replications ----------
            # IDXT2[p, m, t, col] = idx(s = 512m + col*16 + p, tap t)
            #                     = IDXF[h = 4m + col//8, t, a = col%8, b = p]
            # hop1: IDXD2[b, t, h, a] = IDXF[h, t, b, a]  (16 DMAs to DRAM)
            ID2v = IDXD2[:]
            for p16 in range(16):
                eng = nc.sync if (p16 % 2 == 0) else nc.scalar
                eng.dma_start(
                    out=ID2v[p16:p16 + 1, :, :, :].rearrange(
                        "o t h a -> o h t a"),
                    in_=IDXF[:, :, p16, :])
            # hop2: IDXT2[b, m, t, (hl a)] = IDXD2[b, t, 4m+hl, a]  (9 DMAs)
            for t9 in range(T):
                eng = nc.sync if (t9 % 2 == 0) else nc.scalar
                eng.dma_start(
                    out=IDXT2[0:16, :, t9, :],
                    in_=ID2v[:, t9, :, :].rearrange(
                        "b (m hl) a -> b m (hl a)", hl=4))
            # replicate partitions 0-15 -> 16-127 (doubling)
            nc.sync.dma_start(out=IDXT2[16:32], in_=IDXT2[0:16])
            nc.sync.dma_start(out=IDXT2[32:64], in_=IDXT2[0:32])
            nc.sync.dma_start(out=IDXT2[64:128], in_=IDXT2[0:64])

            # ---------- WLIN5 build: contiguous dump + 16 DRAM reorders ----
            # WLIN5[m, ry, rx, t, hl*128 + w] = W4C[4m+hl, ry, rx, t, w]
            WL5v = WLIN5[:]
            W2v = WLIN2[:]
            nc.sync.dma_start(
                out=W2v.rearrange("p a b c d -> p (a b c d)"),
                in_=W4C[:].rearrange("p a b c d -> p (a b c d)"))
            for ry in range(2):
                for rx in range(2):
                    for hl in range(4):
                        eng = nc.sync if (hl % 2 == 0) else nc.scalar
                        eng.dma_start(
                            out=WL5v[:, ry, :, rx, hl * 128:(hl + 1) * 128],
                            in_=W2v[hl:128:4, ry, rx, :, :])
            midcm.__exit__(None, None, None)

            # ================= main loop =================
            XQf = XQ[:].rearrange("n q c -> n (q c)")   # [HW, 256] units
            with tc.tile_pool(name="gl", bufs=3) as gp, \
                 tc.tile_pool(name="wl", bufs=3) as wp, \
                 tc.tile_pool(name="ps2", bufs=2, space="PSUM") as pp2:
                # absorb initial deps into gpsimd queue
                dd2 = mp.tile([128, 1], i16, tag="dd2")
                nc.gpsimd.tensor_copy(out=dd2[:], in_=IDXT2[:, 0, 0, 0:1])

                for m in range(NCHUNK):
                    GQ = gp.tile([128, T, 2, CHUNK], bf16, tag="GQ")
                    WRQ = wp.tile([128, T, 2, CHUNK], bf16, tag="WRQ")
                    # gather: per tap, 512 idx x 4 corners via SDMA engines
                    for tap in range(T):
                        nc.gpsimd.dma_gather(
                            out_ap=GQ[:, tap],
                            in_ap=XQf,
                            idxs_ap=IDXT2[:, m, tap],
                            num_idxs=CHUNK,
                            num_idxs_reg=CHUNK,
                            elem_size=256,
                            transpose=True)
                    # weight broadcast: one DMA per ry-half (from DRAM)
                    for ry in range(2):
                        eng = nc.sync if (ry == 0) else nc.scalar
                        eng.dma_start(
                            out=WRQ[64 * ry:64 * (ry + 1)],
                            in_=WL5v[m, ry].unsqueeze(0)
                                .broadcast_to([64, T, 2, CHUNK]))
                    if debug and m == 0:
                        d_gq = nc.dram_tensor("d_gq", [128, 2 * T * CHUNK], bf16, kind="ExternalOutput")
                        d_wrq = nc.dram_tensor("d_wrq", [128, 2 * T * CHUNK], bf16, kind="ExternalOutput")
                        nc.sync.dma_start(d_gq.ap(), GQ[:].rearrange("p a b c -> p (a b c)"))
                        nc.sync.dma_start(d_wrq.ap(), WRQ[:].rearrange("p a b c -> p (a b c)"))
                    # modulate: P = G * Wrep (in-place into WRQ)
                    WRQf = WRQ[:].rearrange("p a b c -> p (a b c)")
                    nc.vector.tensor_tensor(
                        out=WRQf, in0=GQ[:].rearrange("p a b c -> p (a b c)"),
                        in1=WRQf, op=ALU.mult)
                    # einsum: accumulate 9 taps x 2 rx into PSUM [64, 512]
                    ps = pp2.tile([C, CHUNK], f32, tag="eps")
                    for tap in range(T):
                        for rx in range(2):
                            nc.tensor.matmul(
                                ps[:], lhsT=WK2[:, tap, :],
                                rhs=WRQ[:, tap, rx, :],
                                start=(tap == 0 and rx == 0),
                                stop=(tap == T - 1 and rx == 1))
                    osb = dbp.tile([C, CHUNK], f32, tag="osb")
                    nc.scalar.copy(out=osb[:], in_=ps[:])
                    eng = nc.sync if (m % 2 == 0) else nc.scalar
                    eng.dma_start(
                        out=out_dram.ap()[:, m * CHUNK:(m + 1) * CHUNK],
                        in_=osb[:])

    nc.compile()
    return nc


def _prep_weights(w_offset, b_offset, w_conv):
    w_offset = np.asarray(w_offset, dtype=np.float32)
    w_conv = np.asarray(w_conv, dtype=np.float32)
    b_offset = np.asarray(b_offset, dtype=np.float32)
    # woffT[c, t*27 + j] = w_offset[j, c, ty, tx]
    woffT = w_offset.transpose(2, 3, 1, 0).reshape(T, C, NJ)  # [t, c, j]
    woffT = woffT.transpose(1, 0, 2).reshape(C, T * NJ).copy()
    boff = b_offset.reshape(NJ, 1).copy()
    # wk2[q, t*64 + o] = w_conv[o, q%64, ty, tx]
    wkt = w_conv.transpose(2, 3, 1, 0).reshape(T, C, C)       # [t, c, o]
    wk2 = np.concatenate([wkt, wkt], axis=1)                   # [t, 128, o]
    wk2 = wk2.transpose(1, 0, 2).reshape(128, T * C).astype(np_bf16).copy()
    return woffT, boff, wk2


def _get_runner():
    """Build a persistent jitted shard_map runner for the cached nc (avoids
    per-call retracing that run_bass_via_pjrt pays)."""
    if "runner" in _CACHE:
        return _CACHE["runner"]
    import jax
    import jax.numpy as jnp
    from jax.sharding import Mesh, PartitionSpec, NamedSharding
    from jax.experimental.shard_map import shard_map
    from concourse import bass2jax

    bass2jax.install_neuronx_cc_hook()
    nc = _CACHE["nc"]
    partition_name = (nc.partition_id_tensor.name
                      if nc.partition_id_tensor else None)
    in_names, out_names, out_avals, zero_shapes = [], [], [], []
    for alloc in nc.m.functions[0].allocations:
        if not isinstance(alloc, mybir.MemoryLocationSet):
            continue
        name = alloc.memorylocations[0].name
        if alloc.kind == "ExternalInput":
            if name != partition_name:
                in_names.append(name)
        elif alloc.kind == "ExternalOutput":
            out_names.append(name)
            shape = tuple(alloc.tensor_shape)
            dtype = mybir.dt.np(alloc.dtype)
            out_avals.append(jax.core.ShapedArray(shape, dtype))
            zero_shapes.append((shape, dtype))
    n_params = len(in_names)
    all_in = list(in_names) + list(out_names)
    if partition_name is not None:
        all_in.append(partition_name)
    donate = tuple(range(n_params, n_params + len(out_names)))

    def _body(*args):
        operands = list(args)
        if partition_name is not None:
            operands.append(bass2jax.partition_id_tensor())
        return tuple(bass2jax._bass_exec_p.bind(
            *operands,
            out_avals=tuple(out_avals),
            in_names=tuple(all_in),
            out_names=tuple(out_names),
            lowering_input_output_aliases=(),
            sim_require_finite=True,
            sim_require_nnan=True,
            nc=nc,
        ))

    devices = jax.devices()[:B]
    mesh = Mesh(np.asarray(devices), ("core",))
    in_specs = (PartitionSpec("core"),) * (n_params + len(out_names))
    out_specs = (PartitionSpec("core"),) * len(out_names)
    sharded = jax.jit(
        shard_map(_body, mesh=mesh, in_specs=in_specs, out_specs=out_specs,
                  check_rep=False),
        donate_argnums=donate, keep_unused=True)

    shardings = NamedSharding(mesh, PartitionSpec("core"))
    zeros_fn = jax.jit(
        lambda: tuple(jnp.zeros((B * s[0], *s[1:]), d) for s, d in zero_shapes),
        out_shardings=(shardings,) * len(zero_shapes))

    info = dict(sharded=sharded, zeros_fn=zeros_fn, in_names=in_names,
                out_names=out_names, mesh=mesh, shardings=shardings)
    _CACHE["runner"] = info
    return info


def _concat_inputs(x, woffT, boff, wk2):
    # per-core inputs concatenated along axis 0 (cores share the weights)
    xs = np.ascontiguousarray(x.reshape(B * C, HW))
    return {
        "x_in": xs,
        "woffT_in": np.tile(woffT, (B, 1)),
        "boff_in": np.tile(boff, (B, 1)),
        "wk2_in": np.tile(wk2, (B, 1)),
    }


def kernel(x, w_offset, b_offset, w_conv):
    x = np.asarray(x, dtype=np.float32)
    woffT, boff, wk2 = _prep_weights(w_offset, b_offset, w_conv)
    if "nc" not in _CACHE:
        _CACHE["nc"] = build_nc(num_devices=B)
    r = _get_runner()
    cin = _concat_inputs(x, woffT, boff, wk2)
    args = [cin[n] for n in r["in_names"]]
    zeros = r["zeros_fn"]()
    outs = r["sharded"](*args, *zeros)
    out = np.asarray(outs[r["out_names"].index("out")])
    return out.reshape(B, C, H, W).astype(np.float32)


def bench_exec_ns(inp, reps=16):
    """Amortized per-invocation device time: pipelined repeats with staged
    device inputs (excludes host prep + H2D of the big inputs)."""
    import time
    import jax
    x = np.asarray(inp["x"], dtype=np.float32)
    woffT, boff, wk2 = _prep_weights(inp["w_offset"], inp["b_offset"],
                                     inp["w_conv"])
    if "nc" not in _CACHE:
        _CACHE["nc"] = build_nc(num_devices=B)
    r = _get_runner()
    cin = _concat_inputs(x, woffT, boff, wk2)
    args_host = [cin[n] for n in r["in_names"]]
    args_dev = [jax.device_put(a, r["shardings"]) for a in args_host]
    jax.block_until_ready(args_dev)
    oi = r["out_names"].index("out")
    # warm
    o = r["sharded"](*args_dev, *r["zeros_fn"]())
    jax.block_until_ready(o)
    best = None
    for _ in range(3):
        zs = [r["zeros_fn"]() for _ in range(reps)]
        jax.block_until_ready(zs)
        t0 = time.perf_counter()
        outs = [r["sharded"](*args_dev, *z) for z in zs]
        jax.block_until_ready(outs[-1][oi])
        dt = (time.perf_counter() - t0) / reps
        best = dt if best is None else min(best, dt)
    return best * 1e9
